# revision 15
# baseline (speedup 1.0000x reference)
"""EventTokenizer Trainium2 kernel (8 NeuronCores, SPMD + collectives).

Frames axis T is sharded across 8 cores. Per 128-frame tile a [128, 64]
segment one-hot matrix is built on-device (iota constant vs seg_start)
and used as the stationary matmul operand to accumulate per-segment sums
in PSUM. Entropy needs global energy sums first, so a small early
AllGather provides them; a final AllGather combines all partials and the
tiny [64,553]x[553,256] projection runs replicated on every core.
"""

import math
from contextlib import ExitStack

import numpy as np

import concourse.bass as bass
import concourse.tile as tile
from concourse import bacc
from concourse import mybir
from concourse.bass_utils import run_bass_kernel_spmd
from concourse.masks import make_identity

F32 = mybir.dt.float32
F32R = mybir.dt.float32r
EPS = 1e-6
CONF_TH = 0.3
NCORES = 8
K = 64            # segments
D = 512           # feature dim
NSTAT = 5         # energy, frame_conf, stream_conf x3
KP = K + 1        # + sentinel column
SENTINEL = float(2 ** 30)
TPB = 8           # tiles per feature batch
C1 = 520          # final allgather row width (512 feat + 5 stat + peak + U + pad)

AluOp = mybir.AluOpType
Act = mybir.ActivationFunctionType


def _mk(ap, dims, extra_offset=0):
    """Manual AP with explicit [step, count] dims on the same tensor."""
    return bass.AP(tensor=ap.tensor, offset=ap.offset + extra_offset, ap=dims)


def build(t_shard):
    P = 128
    NT = t_shard // P
    assert t_shard % (P * TPB) == 0
    NB = NT // TPB
    CH = min(32, NT)          # tiles per entropy-product chunk
    NCH = NT // CH

    nc = bacc.Bacc(
        "TRN2",
        target_bir_lowering=False,
        debug=False,
        enable_asserts=False,
        num_devices=NCORES,
    )

    # ---- kernel I/O (per core) ----
    feat_d = nc.dram_tensor("feat", [P, NT, D], F32R, kind="ExternalInput")
    epack_d = nc.dram_tensor("epack", [P, NT, NSTAT], F32R, kind="ExternalInput")
    idx_d = nc.dram_tensor("idx", [P, NT], F32, kind="ExternalInput")
    ss_d = nc.dram_tensor("ssrep", [P, KP], F32, kind="ExternalInput")
    meta_d = nc.dram_tensor("segmeta", [K, 4], F32, kind="ExternalInput")
    ohemb_d = nc.dram_tensor("ohemb", [16, K + 32], F32, kind="ExternalInput")
    wt_d = nc.dram_tensor("wtp", [P, 5, 256], F32, kind="ExternalInput")

    tok_d = nc.dram_tensor("tokens", [K, 256], F32, kind="ExternalOutput")
    sk_d = nc.dram_tensor("sk", [K, 8], F32, kind="ExternalOutput")
    conf_d = nc.dram_tensor("conf", [K, 1], F32, kind="ExternalOutput")
    attn_d = nc.dram_tensor("attn", [K, 1], F32, kind="ExternalOutput")

    # ---- internal DRAM (collective bounce buffers) ----
    ag0_in = nc.dram_tensor("ag0_in", [K, 8], F32)
    ag0_out = nc.dram_tensor("ag0_out", [K * NCORES, 8], F32, addr_space="Shared")
    cb_d = nc.dram_tensor("c_bounce", [K, 1], F32)
    ub_d = nc.dram_tensor("u_bounce", [K, 1], F32)
    ag1_in = nc.dram_tensor("ag1_in", [K, C1], F32)
    ag1_out = nc.dram_tensor("ag1_out", [K * NCORES, C1], F32, addr_space="Shared")

    groups = [list(range(NCORES))]

    with tile.TileContext(nc) as tc, ExitStack() as ctx:
        consts = ctx.enter_context(tc.tile_pool(name="consts", bufs=1))
        featp = ctx.enter_context(tc.tile_pool(name="featp", bufs=3))
        mgen = ctx.enter_context(tc.tile_pool(name="mgen", bufs=2))
        mep = ctx.enter_context(tc.tile_pool(name="mep", bufs=2))
        prodp = ctx.enter_context(tc.tile_pool(name="prodp", bufs=2))
        psum = ctx.enter_context(tc.tile_pool(name="psum", bufs=1, space="PSUM"))
        psum2 = ctx.enter_context(tc.tile_pool(name="psum2", bufs=2, space="PSUM"))

        # ---------- persistent SBUF tiles ----------
        epack_t = consts.tile([P, NT, NSTAT], F32R)
        idx_t = consts.tile([P, NT], F32)
        ss_t = consts.tile([P, KP], F32)
        meta_t = consts.tile([K, 4], F32)
        ohemb_t = consts.tile([16, K + 32], F32)
        wt_t = consts.tile([P, 5, 256], F32)
        ident = consts.tile([P, P], F32)
        oh_all = consts.tile([P, NT, K], F32R)
        maxacc = consts.tile([P, TPB * K], F32)
        crep = consts.tile([P, K], F32)
        cframe = consts.tile([P, NT], F32)
        arg_t = consts.tile([P, NT], F32)
        lg_t = consts.tile([P, NT], F32)
        elg_t = consts.tile([P, NT], F32R)
        pk1 = consts.tile([P, K], F32)
        agbuf = consts.tile([K, C1], F32)
        comb0 = consts.tile([K, NCORES, 8], F32)
        stats0 = consts.tile([K, 8], F32)
        sp0 = consts.tile([K, 1], F32)
        cvec = consts.tile([K, 1], F32)
        comb1 = consts.tile([K, NCORES, C1], F32)
        sums = consts.tile([K, C1], F32)
        z_t = consts.tile([K, 5 * P], F32)
        zt_sb = consts.tile([P, 5, K], F32)
        fin = consts.tile([K, 24], F32)  # scratch columns for the final math
        sk_sb = consts.tile([K, 8], F32)
        s_sb = consts.tile([NSTAT, K], F32)
        u_sb = consts.tile([1, K], F32)
        attn_sb = consts.tile([K, 1], F32)
        zeros_p = consts.tile([P, 1], F32)
        ones_k = consts.tile([K, 1], F32)
        tok_sb = consts.tile([K, 256], F32)

        # ---------- PSUM tiles ----------
        psumF = psum.tile([K, D], F32)
        psumS = psum.tile([NSTAT, K], F32)
        psumU = psum.tile([1, K], F32)
        psumT = psum.tile([K, P], F32)
        psumG = psum.tile([K, 32], F32)
        psumTok = psum.tile([K, 256], F32)

        # ---------- phase 0: constants in ----------
        nc.sync.dma_start(epack_t[:], epack_d.ap())
        nc.sync.dma_start(idx_t[:], idx_d.ap())
        nc.sync.dma_start(ss_t[:], ss_d.ap())
        nc.scalar.dma_start(meta_t[:], meta_d.ap())
        nc.scalar.dma_start(ohemb_t[:], ohemb_d.ap())
        nc.scalar.dma_start(wt_t[:], wt_d.ap())
        make_identity(nc, ident[:])
        nc.vector.memset(maxacc[:], 0.0)
        nc.vector.memset(zeros_p[:], 0.0)
        nc.vector.memset(ones_k[:], 1.0)
        # embedding lookup as one-hot matmul; done here so the instruction
        # carries few semaphore waits (walrus limits waits per LDWEIGHTS)
        nc.tensor.matmul(
            psumG[:], ohemb_t[:, 0:K], ohemb_t[:, K:], start=True, stop=True
        )

        # ---------- phase 1: one-hot segment masks for every tile ----------
        for b in range(NB):
            sl = slice(b * TPB, (b + 1) * TPB)
            ge = mgen.tile([P, TPB, KP], F32)
            idx_sl = idx_t[:, sl]
            in0 = _mk(idx_sl, [*idx_sl.ap, [0, KP]])
            ss_ap = ss_t[:]
            in1 = _mk(ss_ap, [ss_ap.ap[0], [0, TPB], ss_ap.ap[1]])
            nc.vector.tensor_tensor(ge[:], in0, in1, AluOp.is_ge)
            nc.vector.tensor_tensor(
                oh_all[:, sl, :], ge[:, :, 0:K], ge[:, :, 1:KP], AluOp.subtract
            )

        # ---------- phase 2: per-segment stats sums (early, feeds entropy) ----------
        # transposed orientation: stationary = 5-col stats slice (cheap
        # ldweights), moving = one-hot (N=64, even — fp32r requirement)
        for t in range(NT):
            nc.tensor.matmul(
                psumS[:],
                epack_t[:, t, :],
                oh_all[:, t, :],
                start=(t == 0),
                stop=(t == NT - 1),
            )

        # ---------- phase 3: early allgather of stats partials ----------
        nc.vector.memset(agbuf[:, 512:C1], 0.0)
        nc.vector.tensor_copy(s_sb[:], psumS[:])
        nc.tensor.transpose(psumT[0:K, 0:NSTAT], s_sb[:], ident[0:NSTAT, 0:NSTAT])
        nc.vector.tensor_copy(agbuf[:, 512:517], psumT[0:K, 0:NSTAT])
        nc.scalar.dma_start(ag0_in.ap(), agbuf[:, 512:C1])
        nc.gpsimd.collective_compute(
            "AllGather",
            AluOp.bypass,
            replica_groups=groups,
            ins=[ag0_in.ap()],
            outs=[ag0_out.ap()],
        )
        nc.scalar.dma_start(
            comb0[:], ag0_out.ap().rearrange("(r k) c -> k r c", k=K)
        )
        c0 = comb0[:, 0:1, 0:1]
        nc.vector.tensor_reduce(
            stats0[:],
            _mk(c0, [comb0[:].ap[0], [1, 8], [8, NCORES]]),
            axis=mybir.AxisListType.X,
            op=AluOp.add,
        )
        nc.vector.tensor_scalar_add(sp0[:], stats0[:, 0:1], EPS)
        nc.vector.tensor_scalar_mul(cvec[:], sp0[:], EPS)
        nc.scalar.dma_start(cb_d.ap(), cvec[:])
        cbap = cb_d.ap()
        nc.scalar.dma_start(crep[:], _mk(cbap, [[0, P], [1, K]]))

        # ---------- features streaming (first half) ----------
        def feat_batch(b):
            sl = slice(b * TPB, (b + 1) * TPB)
            ft = featp.tile([P, TPB, D], F32R)
            nc.sync.dma_start(ft[:], feat_d.ap()[:, sl, :])
            for j in range(TPB):
                t = b * TPB + j
                nc.tensor.matmul(
                    psumF[:],
                    oh_all[:, t, :],
                    ft[:, j, :],
                    start=(t == 0),
                    stop=(t == NT - 1),
                )

        for b in range(NB // 2):
            feat_batch(b)

        # ---------- entropy pass (needs crep from the early allgather) ----------
        for c in range(NCH):
            sl = slice(c * CH, (c + 1) * CH)
            prod = prodp.tile([P, CH, K], F32)
            cr = crep[:]
            cr_bc = _mk(cr, [cr.ap[0], [0, CH], cr.ap[1]])
            nc.vector.tensor_tensor(prod[:], oh_all[:, sl, :], cr_bc, AluOp.mult)
            nc.vector.tensor_reduce(
                cframe[:, sl], prod[:], axis=mybir.AxisListType.X, op=AluOp.add
            )
        ep0 = epack_t[:, :, 0:1]
        e_view = _mk(ep0, [ep0.ap[0], [NSTAT, NT]])
        nc.vector.tensor_tensor(arg_t[:], e_view, cframe[:], AluOp.add)
        nc.scalar.activation(lg_t[:], arg_t[:], func=Act.Ln, bias=zeros_p[:])
        nc.vector.tensor_tensor(elg_t[:], e_view, lg_t[:], AluOp.mult)

        def u_chunk(ts):
            for t in ts:
                nc.tensor.matmul(
                    psumU[:],
                    elg_t[:, t : t + 1],
                    oh_all[:, t, :],
                    start=(t == 0),
                    stop=(t == NT - 1),
                )

        # ---------- features streaming (second half) ----------
        # U matmuls ride between the last feature batches so the PE stream
        # never stalls long enough to backpressure the feature DMAs
        u_batches = list(range(max(NB // 2, NB - 10), NB))
        u_splits = np.array_split(np.arange(NT), len(u_batches))
        for b in range(NB // 2, NB):
            feat_batch(b)
            if b in u_batches:
                u_chunk(list(u_splits[u_batches.index(b)]))

        # ---------- masked per-partition max of energy ----------
        for b in range(NB):
            sl = slice(b * TPB, (b + 1) * TPB)
            me = mep.tile([P, TPB, K], F32)
            e_sl = epack_t[:, sl, 0:1]
            e_bc = _mk(e_sl, [e_sl.ap[0], e_sl.ap[1], [0, K]])
            nc.vector.tensor_tensor(me[:], oh_all[:, sl, :], e_bc, AluOp.mult)
            ma2 = _mk(maxacc[:], [maxacc[:].ap[0], [K, TPB], [1, K]])
            nc.vector.tensor_tensor(ma2, ma2, me[:], AluOp.max)

        # ---------- peak = cross-partition max ----------
        ma = maxacc[:]
        nc.vector.tensor_reduce(
            pk1[:],
            _mk(ma, [ma.ap[0], [1, K], [K, TPB]]),
            axis=mybir.AxisListType.X,
            op=AluOp.max,
        )
        nc.tensor.transpose(psumT[:], pk1[:], ident[:])
        nc.vector.tensor_reduce(
            agbuf[:, 517:518], psumT[:], axis=mybir.AxisListType.X, op=AluOp.max
        )

        # ---------- phase 8: final allgather of all partials ----------
        nc.vector.tensor_copy(agbuf[:, 0:D], psumF[:])
        nc.vector.tensor_copy(u_sb[:], psumU[:])
        nc.scalar.dma_start(ub_d.ap(), u_sb[:])
        nc.scalar.dma_start(agbuf[:, 518:519], ub_d.ap())
        nc.scalar.dma_start(ag1_in.ap(), agbuf[:])
        nc.gpsimd.collective_compute(
            "AllGather",
            AluOp.bypass,
            replica_groups=groups,
            ins=[ag1_in.ap()],
            outs=[ag1_out.ap()],
        )
        nc.scalar.dma_start(
            comb1[:], ag1_out.ap().rearrange("(r k) c -> k r c", k=K)
        )
        cb1 = comb1[:, 0:1, 0:1]
        nc.vector.tensor_reduce(
            sums[:, 0:519],
            _mk(cb1, [comb1[:].ap[0], [1, 519], [C1, NCORES]]),
            axis=mybir.AxisListType.X,
            op=AluOp.add,
        )
        pk_in = comb1[:, 0:1, 517:518]
        nc.vector.tensor_reduce(
            sums[:, 517:518],
            _mk(pk_in, [comb1[:].ap[0], [C1, NCORES]]),
            axis=mybir.AxisListType.X,
            op=AluOp.max,
        )

        # ---------- phase 9: replicated final math ----------
        # fin columns: 0 seglen, 1 invlen, 2 S', 3 invS, 4 lnS, 5 dur, 6 lat,
        #              7 meanmot, 8 confmean, 9..11 vismean, 12 t1, 13 t2, 14 H
        seglen = fin[:, 0:1]
        invlen = fin[:, 1:2]
        sp = fin[:, 2:3]
        invS = fin[:, 3:4]
        lnS = fin[:, 4:5]
        dur = fin[:, 5:6]
        lat = fin[:, 6:7]
        meanmot = fin[:, 7:8]
        confmean = fin[:, 8:9]
        vismean = fin[:, 9:12]
        t1 = fin[:, 12:13]
        t2 = fin[:, 13:14]
        hh = fin[:, 14:15]

        start_c = meta_t[:, 0:1]
        end_c = meta_t[:, 1:2]
        invfps = meta_t[:, 2:3]

        nc.vector.tensor_tensor(seglen, end_c, start_c, AluOp.subtract)
        nc.vector.reciprocal(invlen, seglen)
        nc.vector.tensor_scalar_add(sp, sums[:, 512:513], EPS)
        nc.vector.reciprocal(invS, sp)
        nc.scalar.activation(lnS, sp, func=Act.Ln, bias=zeros_p[0:K, :])
        nc.vector.tensor_tensor(dur, seglen, invfps, AluOp.mult)
        nc.vector.tensor_tensor(lat, start_c, invfps, AluOp.mult)
        nc.vector.tensor_tensor(meanmot, sums[:, 512:513], invlen, AluOp.mult)
        nc.vector.tensor_tensor(confmean, sums[:, 513:514], invlen, AluOp.mult)
        il = invlen
        il_bc = _mk(il, [il.ap[0], [0, 3]])
        nc.vector.tensor_tensor(vismean, sums[:, 514:517], il_bc, AluOp.mult)
        nc.vector.tensor_tensor(t1, lnS, sums[:, 512:513], AluOp.mult)
        nc.vector.tensor_tensor(t2, t1, sums[:, 518:519], AluOp.subtract)
        nc.vector.tensor_tensor(hh, t2, invS, AluOp.mult)

        spre = consts.tile([K, 8], F32)
        nc.vector.tensor_copy(spre[:, 0:1], dur)
        nc.vector.tensor_copy(spre[:, 1:2], lat)
        nc.vector.tensor_copy(spre[:, 2:3], meanmot)
        nc.vector.tensor_copy(spre[:, 3:4], sums[:, 517:518])
        nc.vector.tensor_copy(spre[:, 4:5], hh)
        nc.vector.tensor_copy(spre[:, 5:8], vismean)
        nc.scalar.activation(sk_sb[:], spre[:], func=Act.Ln, bias=ones_k[:], scale=1.0)

        nc.vector.tensor_scalar(
            attn_sb[:], confmean, CONF_TH, None, op0=AluOp.is_ge
        )

        # z = [e_k | s_k | g_k | 1 | 0-pad]
        nc.vector.memset(z_t[:], 0.0)
        nc.vector.tensor_scalar(
            z_t[:, 0:D], sums[:, 0:D], invlen, None, op0=AluOp.mult
        )
        nc.vector.tensor_copy(z_t[:, D : D + 8], sk_sb[:])
        nc.vector.tensor_copy(z_t[:, 520:552], psumG[:])
        nc.vector.memset(z_t[:, 552:553], 1.0)

        for c in range(5):
            pz = psum2.tile([P, K], F32)
            nc.tensor.transpose(
                pz[:], z_t[:, c * P : (c + 1) * P], ident[0:K, 0:K]
            )
            nc.vector.tensor_copy(zt_sb[:, c, :], pz[:])
        for c in range(5):
            nc.tensor.matmul(
                psumTok[:],
                zt_sb[:, c, :],
                wt_t[:, c, :],
                start=(c == 0),
                stop=(c == 4),
            )
        nc.vector.tensor_copy(tok_sb[:], psumTok[:])

        nc.scalar.dma_start(tok_d.ap(), tok_sb[:])
        nc.scalar.dma_start(sk_d.ap(), sk_sb[:])
        nc.scalar.dma_start(conf_d.ap(), confmean)
        nc.scalar.dma_start(attn_d.ap(), attn_sb[:])

    nc.compile()
    return nc


def prep_inputs(features, energy, frame_conf, stream_conf, seg_start, seg_end,
                event_type_id, emb, W, bias, fps):
    """Host-side sharding + layout marshaling. Returns per-core input dicts."""
    features = np.ascontiguousarray(np.asarray(features, np.float32))
    energy = np.asarray(energy, np.float32)
    frame_conf = np.asarray(frame_conf, np.float32)
    stream_conf = np.asarray(stream_conf, np.float32)
    seg_start = np.asarray(seg_start)
    seg_end = np.asarray(seg_end)
    event_type_id = np.asarray(event_type_id)
    emb = np.ascontiguousarray(np.asarray(emb, np.float32))
    W = np.asarray(W, np.float32)
    bias = np.asarray(bias, np.float32)
    inv_fps = 1.0 / float(np.asarray(fps))

    T = energy.shape[0]
    t_shard = T // NCORES
    P = 128
    NT = t_shard // P

    ss_ext = np.concatenate([seg_start.astype(np.float32), [SENTINEL]])
    ssrep = np.ascontiguousarray(np.tile(ss_ext[None, :], (P, 1)))

    meta = np.zeros((K, 4), np.float32)
    meta[:, 0] = seg_start.astype(np.float32)
    meta[:, 1] = seg_end.astype(np.float32)
    meta[:, 2] = inv_fps

    ohemb = np.zeros((16, K + 32), np.float32)
    ohemb[event_type_id.astype(np.int64), np.arange(K)] = 1.0
    ohemb[:, K:] = emb

    wt = np.zeros((5 * P, W.shape[0]), np.float32)
    wt[: W.shape[1], :] = W.T
    wt[552, :] = bias
    wtp = np.ascontiguousarray(wt.reshape(5, P, W.shape[0]).transpose(1, 0, 2))

    pk = np.stack([energy, frame_conf], axis=1)
    epack_full = np.concatenate([pk, stream_conf.astype(np.float32)], axis=1)

    in_maps = []
    for c in range(NCORES):
        lo, hi = c * t_shard, (c + 1) * t_shard
        ep = np.ascontiguousarray(
            epack_full[lo:hi].reshape(NT, P, NSTAT).transpose(1, 0, 2)
        )
        idx = np.ascontiguousarray(
            (lo + np.arange(t_shard, dtype=np.float32)).reshape(NT, P).T
        )
        in_maps.append(
            dict(
                feat=np.ascontiguousarray(
                    features[lo:hi].reshape(NT, P, D).transpose(1, 0, 2)
                ),
                epack=ep,
                idx=idx,
                ssrep=ssrep,
                segmeta=meta,
                ohemb=ohemb,
                wtp=wtp,
                )
        )
    return in_maps


_BUILT = {}


def kernel(features, energy, frame_conf, stream_conf, seg_start, seg_end,
           event_type_id, emb, W, bias, fps):
    energy_np = np.asarray(energy)
    T = energy_np.shape[0]
    t_shard = T // NCORES

    if t_shard not in _BUILT:
        _BUILT[t_shard] = build(t_shard)
    nc = _BUILT[t_shard]

    in_maps = prep_inputs(features, energy, frame_conf, stream_conf, seg_start,
                          seg_end, event_type_id, emb, W, bias, fps)
    res = run_bass_kernel_spmd(nc, in_maps, core_ids=list(range(NCORES)))
    r0 = res.results[0]

    tokens = np.asarray(r0["tokens"], np.float32)
    sk = np.asarray(r0["sk"], np.float32)
    conf = np.asarray(r0["conf"], np.float32)[:, 0]
    attn = np.asarray(r0["attn"], np.float32)[:, 0] > 0.5
    etid = np.asarray(event_type_id)
    return tokens, attn, etid, conf, sk


# revision 16
# speedup vs baseline: 1.0898x; 1.0898x over previous
"""EventTokenizer Trainium2 kernel (8 NeuronCores, SPMD + collectives).

Frames axis T is sharded across 8 cores. Per 128-frame tile a [128, 64]
segment one-hot matrix is built on-device (iota constant vs seg_start)
and used as the stationary matmul operand to accumulate per-segment sums
in PSUM. Entropy needs global energy sums first, so a small early
AllGather provides them; a final AllGather combines all partials and the
tiny [64,553]x[553,256] projection runs replicated on every core.
"""

import math
from contextlib import ExitStack

import numpy as np

import concourse.bass as bass
import concourse.tile as tile
from concourse import bacc
from concourse import mybir
from concourse.bass_utils import run_bass_kernel_spmd
from concourse.masks import make_identity

F32 = mybir.dt.float32
F32R = mybir.dt.float32r
EPS = 1e-6
CONF_TH = 0.3
NCORES = 8
K = 64            # segments
D = 512           # feature dim
NSTAT = 5         # energy, frame_conf, stream_conf x3
KP = K + 1        # + sentinel column
SENTINEL = float(2 ** 30)
TPB = 8           # tiles per feature batch
C1 = 520          # final allgather row width (512 feat + 5 stat + peak + U + pad)

AluOp = mybir.AluOpType
Act = mybir.ActivationFunctionType


def _mk(ap, dims, extra_offset=0):
    """Manual AP with explicit [step, count] dims on the same tensor."""
    return bass.AP(tensor=ap.tensor, offset=ap.offset + extra_offset, ap=dims)


def build(t_shard):
    P = 128
    NT = t_shard // P
    assert t_shard % (P * TPB) == 0
    NB = NT // TPB
    CH = min(32, NT)          # tiles per entropy-product chunk
    NCH = NT // CH

    nc = bacc.Bacc(
        "TRN2",
        target_bir_lowering=False,
        debug=False,
        enable_asserts=False,
        num_devices=NCORES,
    )

    # ---- kernel I/O (per core) ----
    feat_d = nc.dram_tensor("feat", [P, NT, D], F32R, kind="ExternalInput")
    epack_d = nc.dram_tensor("epack", [P, NT, NSTAT], F32R, kind="ExternalInput")
    idx_d = nc.dram_tensor("idx", [P, NT], F32, kind="ExternalInput")
    ss_d = nc.dram_tensor("ssrep", [P, KP], F32, kind="ExternalInput")
    meta_d = nc.dram_tensor("segmeta", [K, 4], F32, kind="ExternalInput")
    ohemb_d = nc.dram_tensor("ohemb", [16, K + 32], F32, kind="ExternalInput")
    wt_d = nc.dram_tensor("wtp", [P, 5, 256], F32, kind="ExternalInput")

    tok_d = nc.dram_tensor("tokens", [K, 256], F32, kind="ExternalOutput")
    sk_d = nc.dram_tensor("sk", [K, 8], F32, kind="ExternalOutput")
    conf_d = nc.dram_tensor("conf", [K, 1], F32, kind="ExternalOutput")
    attn_d = nc.dram_tensor("attn", [K, 1], F32, kind="ExternalOutput")

    # ---- internal DRAM (collective bounce buffers) ----
    ar0_in = nc.dram_tensor("ar0_in", [8, K], F32)
    ar0_out = nc.dram_tensor("ar0_out", [8, K], F32, addr_space="Shared")
    ub_d = nc.dram_tensor("u_bounce", [K, 1], F32)
    ag1_in = nc.dram_tensor("ag1_in", [K, C1], F32)
    ag1_out = nc.dram_tensor("ag1_out", [K * NCORES, C1], F32, addr_space="Shared")

    groups = [list(range(NCORES))]

    with tile.TileContext(nc) as tc, ExitStack() as ctx:
        consts = ctx.enter_context(tc.tile_pool(name="consts", bufs=1))
        featp = ctx.enter_context(tc.tile_pool(name="featp", bufs=3))
        mgen = ctx.enter_context(tc.tile_pool(name="mgen", bufs=2))
        mep = ctx.enter_context(tc.tile_pool(name="mep", bufs=2))
        prodp = ctx.enter_context(tc.tile_pool(name="prodp", bufs=2))
        psum = ctx.enter_context(tc.tile_pool(name="psum", bufs=1, space="PSUM"))
        psum2 = ctx.enter_context(tc.tile_pool(name="psum2", bufs=2, space="PSUM"))

        # ---------- persistent SBUF tiles ----------
        epack_t = consts.tile([P, NT, NSTAT], F32R)
        idx_t = consts.tile([P, NT], F32)
        ss_t = consts.tile([P, KP], F32)
        meta_t = consts.tile([K, 4], F32)
        ohemb_t = consts.tile([16, K + 32], F32)
        wt_t = consts.tile([P, 5, 256], F32)
        ident = consts.tile([P, P], F32)
        oh_all = consts.tile([P, NT, K], F32R)
        maxacc = consts.tile([P, TPB * K], F32)
        crep = consts.tile([P, K], F32)
        cframe = consts.tile([P, NT], F32)
        arg_t = consts.tile([P, NT], F32)
        lg_t = consts.tile([P, NT], F32)
        elg_t = consts.tile([P, NT], F32R)
        pk1 = consts.tile([P, K], F32)
        agbuf = consts.tile([K, C1], F32)
        ar0st = consts.tile([8, K], F32)
        comb1 = consts.tile([K, NCORES, C1], F32)
        sums = consts.tile([K, C1], F32)
        z_t = consts.tile([K, 5 * P], F32)
        zt_sb = consts.tile([P, 5, K], F32)
        fin = consts.tile([K, 24], F32)  # scratch columns for the final math
        sk_sb = consts.tile([K, 8], F32)
        s_sb = consts.tile([NSTAT, K], F32)
        u_sb = consts.tile([1, K], F32)
        attn_sb = consts.tile([K, 1], F32)
        zeros_p = consts.tile([P, 1], F32)
        ones_k = consts.tile([K, 1], F32)
        tok_sb = consts.tile([K, 256], F32)

        # ---------- PSUM tiles ----------
        psumF = psum.tile([K, D], F32)
        psumS = psum.tile([NSTAT, K], F32)
        psumU = psum.tile([1, K], F32)
        psumT = psum.tile([K, P], F32)
        psumG = psum.tile([K, 32], F32)
        psumTok = psum.tile([K, 256], F32)

        # ---------- phase 0: constants in ----------
        nc.sync.dma_start(epack_t[:], epack_d.ap())
        nc.sync.dma_start(idx_t[:], idx_d.ap())
        nc.sync.dma_start(ss_t[:], ss_d.ap())
        nc.scalar.dma_start(meta_t[:], meta_d.ap())
        nc.scalar.dma_start(ohemb_t[:], ohemb_d.ap())
        nc.scalar.dma_start(wt_t[:], wt_d.ap())
        make_identity(nc, ident[:])
        nc.vector.memset(maxacc[:], 0.0)
        nc.vector.memset(zeros_p[:], 0.0)
        nc.vector.memset(ones_k[:], 1.0)
        # embedding lookup as one-hot matmul; done here so the instruction
        # carries few semaphore waits (walrus limits waits per LDWEIGHTS)
        nc.tensor.matmul(
            psumG[:], ohemb_t[:, 0:K], ohemb_t[:, K:], start=True, stop=True
        )

        # ---------- phase 1: one-hot segment masks for every tile ----------
        for b in range(NB):
            sl = slice(b * TPB, (b + 1) * TPB)
            ge = mgen.tile([P, TPB, KP], F32)
            idx_sl = idx_t[:, sl]
            in0 = _mk(idx_sl, [*idx_sl.ap, [0, KP]])
            ss_ap = ss_t[:]
            in1 = _mk(ss_ap, [ss_ap.ap[0], [0, TPB], ss_ap.ap[1]])
            nc.vector.tensor_tensor(ge[:], in0, in1, AluOp.is_ge)
            nc.vector.tensor_tensor(
                oh_all[:, sl, :], ge[:, :, 0:K], ge[:, :, 1:KP], AluOp.subtract
            )

        # ---------- phase 2: per-segment stats sums (early, feeds entropy) ----------
        # transposed orientation: stationary = 5-col stats slice (cheap
        # ldweights), moving = one-hot (N=64, even — fp32r requirement)
        for t in range(NT):
            nc.tensor.matmul(
                psumS[:],
                epack_t[:, t, :],
                oh_all[:, t, :],
                start=(t == 0),
                stop=(t == NT - 1),
            )

        # ---------- phase 3: early AllReduce of stats partials ----------
        # stage is [8, 64] (stats kind x segment) so every consumer reads
        # contiguous rows; the CCE does the cross-core sum, no local combine
        nc.vector.memset(agbuf[:, 512:C1], 0.0)
        nc.vector.memset(ar0st[:], 0.0)
        nc.vector.tensor_copy(s_sb[:], psumS[:])
        nc.vector.tensor_copy(ar0st[0:NSTAT, :], s_sb[:])
        nc.tensor.transpose(psumT[0:K, 0:NSTAT], s_sb[:], ident[0:NSTAT, 0:NSTAT])
        nc.vector.tensor_copy(agbuf[:, 512:517], psumT[0:K, 0:NSTAT])
        nc.scalar.dma_start(ar0_in.ap(), ar0st[:])
        nc.gpsimd.collective_compute(
            "AllReduce",
            AluOp.add,
            replica_groups=groups,
            ins=[ar0_in.ap()],
            outs=[ar0_out.ap()],
        )
        # crep rows = global sum_e broadcast to all partitions
        ar0ap = ar0_out.ap()
        nc.scalar.dma_start(crep[:], _mk(ar0ap, [[0, P], [1, K]]))

        # ---------- features streaming (first half) ----------
        def feat_batch(b):
            sl = slice(b * TPB, (b + 1) * TPB)
            ft = featp.tile([P, TPB, D], F32R)
            nc.sync.dma_start(ft[:], feat_d.ap()[:, sl, :])
            for j in range(TPB):
                t = b * TPB + j
                nc.tensor.matmul(
                    psumF[:],
                    oh_all[:, t, :],
                    ft[:, j, :],
                    start=(t == 0),
                    stop=(t == NT - 1),
                )

        for b in range(NB // 2):
            feat_batch(b)

        # ---------- entropy pass (needs crep from the early allgather) ----------
        for c in range(NCH):
            sl = slice(c * CH, (c + 1) * CH)
            prod = prodp.tile([P, CH, K], F32)
            cr = crep[:]
            cr_bc = _mk(cr, [cr.ap[0], [0, CH], cr.ap[1]])
            nc.vector.tensor_tensor(prod[:], oh_all[:, sl, :], cr_bc, AluOp.mult)
            nc.vector.tensor_reduce(
                cframe[:, sl], prod[:], axis=mybir.AxisListType.X, op=AluOp.add
            )
        ep0 = epack_t[:, :, 0:1]
        e_view = _mk(ep0, [ep0.ap[0], [NSTAT, NT]])
        nc.vector.scalar_tensor_tensor(
            arg_t[:], cframe[:], EPS, e_view, op0=AluOp.mult, op1=AluOp.add
        )
        nc.scalar.activation(lg_t[:], arg_t[:], func=Act.Ln, bias=zeros_p[:])
        nc.vector.tensor_tensor(elg_t[:], e_view, lg_t[:], AluOp.mult)

        def u_chunk(ts):
            for t in ts:
                nc.tensor.matmul(
                    psumU[:],
                    elg_t[:, t : t + 1],
                    oh_all[:, t, :],
                    start=(t == 0),
                    stop=(t == NT - 1),
                )

        # ---------- features streaming (second half) ----------
        # U matmuls ride between the last feature batches so the PE stream
        # never stalls long enough to backpressure the feature DMAs
        u_batches = list(range(max(NB // 2, NB - 10), NB))
        u_splits = np.array_split(np.arange(NT), len(u_batches))
        for b in range(NB // 2, NB):
            feat_batch(b)
            if b in u_batches:
                u_chunk(list(u_splits[u_batches.index(b)]))

        # ---------- masked per-partition max of energy ----------
        for b in range(NB):
            sl = slice(b * TPB, (b + 1) * TPB)
            me = mep.tile([P, TPB, K], F32)
            e_sl = epack_t[:, sl, 0:1]
            e_bc = _mk(e_sl, [e_sl.ap[0], e_sl.ap[1], [0, K]])
            nc.vector.tensor_tensor(me[:], oh_all[:, sl, :], e_bc, AluOp.mult)
            ma2 = _mk(maxacc[:], [maxacc[:].ap[0], [K, TPB], [1, K]])
            nc.vector.tensor_tensor(ma2, ma2, me[:], AluOp.max)

        # ---------- peak = cross-partition max ----------
        ma = maxacc[:]
        nc.vector.tensor_reduce(
            pk1[:],
            _mk(ma, [ma.ap[0], [1, K], [K, TPB]]),
            axis=mybir.AxisListType.X,
            op=AluOp.max,
        )
        nc.tensor.transpose(psumT[:], pk1[:], ident[:])
        nc.vector.tensor_reduce(
            agbuf[:, 517:518], psumT[:], axis=mybir.AxisListType.X, op=AluOp.max
        )

        # ---------- phase 8: final allgather of all partials ----------
        nc.vector.tensor_copy(agbuf[:, 0:D], psumF[:])
        nc.vector.tensor_copy(u_sb[:], psumU[:])
        nc.scalar.dma_start(ub_d.ap(), u_sb[:])
        nc.scalar.dma_start(agbuf[:, 518:519], ub_d.ap())
        nc.scalar.dma_start(ag1_in.ap(), agbuf[:])
        nc.gpsimd.collective_compute(
            "AllGather",
            AluOp.bypass,
            replica_groups=groups,
            ins=[ag1_in.ap()],
            outs=[ag1_out.ap()],
        )
        comb1_src = ag1_out.ap().rearrange("(r k) c -> k r c", k=K)
        H1 = C1 // 2
        nc.sync.dma_start(comb1[:, :, 0:H1], comb1_src[:, :, 0:H1])
        nc.scalar.dma_start(comb1[:, :, H1:], comb1_src[:, :, H1:])
        cb1 = comb1[:, 0:1, 0:1]
        nc.vector.tensor_reduce(
            sums[:, 0:519],
            _mk(cb1, [comb1[:].ap[0], [1, 519], [C1, NCORES]]),
            axis=mybir.AxisListType.X,
            op=AluOp.add,
        )
        pk_in = comb1[:, 0:1, 517:518]
        nc.vector.tensor_reduce(
            sums[:, 517:518],
            _mk(pk_in, [comb1[:].ap[0], [C1, NCORES]]),
            axis=mybir.AxisListType.X,
            op=AluOp.max,
        )

        # ---------- phase 9: replicated final math ----------
        # fin columns: 0 seglen, 1 invlen, 2 S', 3 invS, 4 lnS, 5 dur, 6 lat,
        #              7 meanmot, 8 confmean, 9..11 vismean, 12 t1, 13 t2, 14 H
        seglen = fin[:, 0:1]
        invlen = fin[:, 1:2]
        sp = fin[:, 2:3]
        invS = fin[:, 3:4]
        lnS = fin[:, 4:5]
        dur = fin[:, 5:6]
        lat = fin[:, 6:7]
        meanmot = fin[:, 7:8]
        confmean = fin[:, 8:9]
        vismean = fin[:, 9:12]
        t1 = fin[:, 12:13]
        t2 = fin[:, 13:14]
        hh = fin[:, 14:15]

        start_c = meta_t[:, 0:1]
        end_c = meta_t[:, 1:2]
        invfps = meta_t[:, 2:3]

        nc.vector.tensor_tensor(seglen, end_c, start_c, AluOp.subtract)
        nc.vector.reciprocal(invlen, seglen)
        nc.vector.tensor_scalar_add(sp, sums[:, 512:513], EPS)
        nc.vector.reciprocal(invS, sp)
        nc.scalar.activation(lnS, sp, func=Act.Ln, bias=zeros_p[0:K, :])
        nc.vector.tensor_tensor(dur, seglen, invfps, AluOp.mult)
        nc.vector.tensor_tensor(lat, start_c, invfps, AluOp.mult)
        nc.vector.tensor_tensor(meanmot, sums[:, 512:513], invlen, AluOp.mult)
        nc.vector.tensor_tensor(confmean, sums[:, 513:514], invlen, AluOp.mult)
        il = invlen
        il_bc = _mk(il, [il.ap[0], [0, 3]])
        nc.vector.tensor_tensor(vismean, sums[:, 514:517], il_bc, AluOp.mult)
        nc.vector.tensor_tensor(t1, lnS, sums[:, 512:513], AluOp.mult)
        nc.vector.tensor_tensor(t2, t1, sums[:, 518:519], AluOp.subtract)
        nc.vector.tensor_tensor(hh, t2, invS, AluOp.mult)

        spre = consts.tile([K, 8], F32)
        nc.vector.tensor_copy(spre[:, 0:1], dur)
        nc.vector.tensor_copy(spre[:, 1:2], lat)
        nc.vector.tensor_copy(spre[:, 2:3], meanmot)
        nc.vector.tensor_copy(spre[:, 3:4], sums[:, 517:518])
        nc.vector.tensor_copy(spre[:, 4:5], hh)
        nc.vector.tensor_copy(spre[:, 5:8], vismean)
        nc.scalar.activation(sk_sb[:], spre[:], func=Act.Ln, bias=ones_k[:], scale=1.0)

        nc.vector.tensor_scalar(
            attn_sb[:], confmean, CONF_TH, None, op0=AluOp.is_ge
        )

        # z = [e_k | s_k | g_k | 1 | 0-pad]
        nc.vector.memset(z_t[:], 0.0)
        nc.vector.tensor_scalar(
            z_t[:, 0:D], sums[:, 0:D], invlen, None, op0=AluOp.mult
        )
        nc.vector.tensor_copy(z_t[:, D : D + 8], sk_sb[:])
        nc.vector.tensor_copy(z_t[:, 520:552], psumG[:])
        nc.vector.memset(z_t[:, 552:553], 1.0)

        for c in range(5):
            pz = psum2.tile([P, K], F32)
            nc.tensor.transpose(
                pz[:], z_t[:, c * P : (c + 1) * P], ident[0:K, 0:K]
            )
            nc.vector.tensor_copy(zt_sb[:, c, :], pz[:])
        for c in range(5):
            nc.tensor.matmul(
                psumTok[:],
                zt_sb[:, c, :],
                wt_t[:, c, :],
                start=(c == 0),
                stop=(c == 4),
            )
        nc.vector.tensor_copy(tok_sb[:], psumTok[:])

        nc.scalar.dma_start(tok_d.ap(), tok_sb[:])
        nc.scalar.dma_start(sk_d.ap(), sk_sb[:])
        nc.scalar.dma_start(conf_d.ap(), confmean)
        nc.scalar.dma_start(attn_d.ap(), attn_sb[:])

    nc.compile()
    return nc


def prep_inputs(features, energy, frame_conf, stream_conf, seg_start, seg_end,
                event_type_id, emb, W, bias, fps):
    """Host-side sharding + layout marshaling. Returns per-core input dicts."""
    features = np.ascontiguousarray(np.asarray(features, np.float32))
    energy = np.asarray(energy, np.float32)
    frame_conf = np.asarray(frame_conf, np.float32)
    stream_conf = np.asarray(stream_conf, np.float32)
    seg_start = np.asarray(seg_start)
    seg_end = np.asarray(seg_end)
    event_type_id = np.asarray(event_type_id)
    emb = np.ascontiguousarray(np.asarray(emb, np.float32))
    W = np.asarray(W, np.float32)
    bias = np.asarray(bias, np.float32)
    inv_fps = 1.0 / float(np.asarray(fps))

    T = energy.shape[0]
    t_shard = T // NCORES
    P = 128
    NT = t_shard // P

    ss_ext = np.concatenate([seg_start.astype(np.float32), [SENTINEL]])
    ssrep = np.ascontiguousarray(np.tile(ss_ext[None, :], (P, 1)))

    meta = np.zeros((K, 4), np.float32)
    meta[:, 0] = seg_start.astype(np.float32)
    meta[:, 1] = seg_end.astype(np.float32)
    meta[:, 2] = inv_fps

    ohemb = np.zeros((16, K + 32), np.float32)
    ohemb[event_type_id.astype(np.int64), np.arange(K)] = 1.0
    ohemb[:, K:] = emb

    wt = np.zeros((5 * P, W.shape[0]), np.float32)
    wt[: W.shape[1], :] = W.T
    wt[552, :] = bias
    wtp = np.ascontiguousarray(wt.reshape(5, P, W.shape[0]).transpose(1, 0, 2))

    pk = np.stack([energy, frame_conf], axis=1)
    epack_full = np.concatenate([pk, stream_conf.astype(np.float32)], axis=1)

    in_maps = []
    for c in range(NCORES):
        lo, hi = c * t_shard, (c + 1) * t_shard
        ep = np.ascontiguousarray(
            epack_full[lo:hi].reshape(NT, P, NSTAT).transpose(1, 0, 2)
        )
        idx = np.ascontiguousarray(
            (lo + np.arange(t_shard, dtype=np.float32)).reshape(NT, P).T
        )
        in_maps.append(
            dict(
                feat=np.ascontiguousarray(
                    features[lo:hi].reshape(NT, P, D).transpose(1, 0, 2)
                ),
                epack=ep,
                idx=idx,
                ssrep=ssrep,
                segmeta=meta,
                ohemb=ohemb,
                wtp=wtp,
                )
        )
    return in_maps


_BUILT = {}


def kernel(features, energy, frame_conf, stream_conf, seg_start, seg_end,
           event_type_id, emb, W, bias, fps):
    energy_np = np.asarray(energy)
    T = energy_np.shape[0]
    t_shard = T // NCORES

    if t_shard not in _BUILT:
        _BUILT[t_shard] = build(t_shard)
    nc = _BUILT[t_shard]

    in_maps = prep_inputs(features, energy, frame_conf, stream_conf, seg_start,
                          seg_end, event_type_id, emb, W, bias, fps)
    res = run_bass_kernel_spmd(nc, in_maps, core_ids=list(range(NCORES)))
    r0 = res.results[0]

    tokens = np.asarray(r0["tokens"], np.float32)
    sk = np.asarray(r0["sk"], np.float32)
    conf = np.asarray(r0["conf"], np.float32)[:, 0]
    attn = np.asarray(r0["attn"], np.float32)[:, 0] > 0.5
    etid = np.asarray(event_type_id)
    return tokens, attn, etid, conf, sk
